# revision 3
# baseline (speedup 1.0000x reference)
"""Trainium2 Bass kernel for nn_Attention_28930899706081 (sparse_attention).

Reference computation:
  k1 = l2norm_c(Wqk @ fmap1), k2 = l2norm_c(Wqk @ fmap2), q = l2norm_c(Wqk @ dmap)
  sim_i = q^T k_i per batch  -> [b, n, n] with n = h*w = 4096
  attn_i = softmax(sim_i, axis=-1)[:, None]  -> [b, 1, n, n]
  returns (attn1, attn2)

Sharding: 8 cores; core i handles batch b = i//4 and query-row block r = i%4
(1024 of 4096 rows). Each core computes the full normalized K for its batch
(recompute instead of collectives) and its row block of both sims + softmax.

Compute dtype bf16 (fp32 accumulation in PSUM); |sim| <= 1 because q/k are
unit vectors, so softmax needs no max subtraction. Row sums come from the
ScalarE activation accumulator fused with exp. Output is written bf16 and
upcast to f32 on the host.
"""

import numpy as np
import ml_dtypes

B, C, H, W, D = 2, 256, 64, 64, 128
N = H * W  # 4096
QBLK = N // 4  # 1024 query rows per core
N_CORES = 8

_cached = {}


def _build():
    import concourse.mybir as mybir
    import concourse.tile as tile
    from concourse import bacc
    from contextlib import ExitStack

    f32 = mybir.dt.float32
    bf16 = mybir.dt.bfloat16
    AF = mybir.ActivationFunctionType

    nc = bacc.Bacc(
        "TRN2",
        target_bir_lowering=False,
        debug=False,
        enable_asserts=True,
        num_devices=N_CORES,
    )

    f1_ext = nc.dram_tensor("f1", [C, N], bf16, kind="ExternalInput").ap()
    f2_ext = nc.dram_tensor("f2", [C, N], bf16, kind="ExternalInput").ap()
    xq_ext = nc.dram_tensor("xq", [C, QBLK], bf16, kind="ExternalInput").ap()
    wqkT_ext = nc.dram_tensor("wqkT", [C, D], bf16, kind="ExternalInput").ap()
    out_ext = nc.dram_tensor("out", [2, QBLK, N], bf16, kind="ExternalOutput").ap()

    PCH = 512  # proj matmul free-dim chunk
    NCH = 1024  # norm/sim/exp chunk

    with tile.TileContext(nc) as tc, ExitStack() as ctx:
        consts = ctx.enter_context(tc.tile_pool(name="consts", bufs=1))
        xin = ctx.enter_context(tc.tile_pool(name="xin", bufs=4))
        ybf_pool = ctx.enter_context(tc.tile_pool(name="ybf", bufs=2))
        ysq_pool = ctx.enter_context(tc.tile_pool(name="ysq", bufs=2))
        lnn2_pool = ctx.enter_context(tc.tile_pool(name="lnn2", bufs=2))
        rk_pool = ctx.enter_context(tc.tile_pool(name="rk", bufs=2))
        kn_pool = ctx.enter_context(tc.tile_pool(name="kn", bufs=1))
        e_pool = ctx.enter_context(tc.tile_pool(name="epool", bufs=8))
        attn_pool = ctx.enter_context(tc.tile_pool(name="attn", bufs=3))
        stat_pool = ctx.enter_context(tc.tile_pool(name="stat", bufs=4))
        proj_psum = ctx.enter_context(
            tc.tile_pool(name="proj_psum", bufs=2, space="PSUM")
        )
        n2_psum = ctx.enter_context(tc.tile_pool(name="n2_psum", bufs=1, space="PSUM"))
        sim_psum = ctx.enter_context(
            tc.tile_pool(name="sim_psum", bufs=2, space="PSUM")
        )

        # constants
        wqkT_sb = [consts.tile([128, D], bf16, tag=f"wqkT{k}", name=f"wqkT{k}") for k in range(2)]
        nc.sync.dma_start(out=wqkT_sb[0][:], in_=wqkT_ext[0:128, :])
        nc.sync.dma_start(out=wqkT_sb[1][:], in_=wqkT_ext[128:256, :])
        ones_sb = consts.tile([128, 128], bf16, tag="ones", name="ones")
        nc.vector.memset(ones_sb[:], 1.0)

        def phase_a(x_ext, ncols, tagbase):
            """project + l2-normalize columns: returns normalized [128, ncols] bf16."""
            x_lo = xin.tile([128, N], bf16, tag="xin", name="x_lo")[:, :ncols]
            x_hi = xin.tile([128, N], bf16, tag="xin", name="x_hi")[:, :ncols]
            nc.sync.dma_start(out=x_lo, in_=x_ext[0:128, :])
            nc.sync.dma_start(out=x_hi, in_=x_ext[128:256, :])

            y_bf = ybf_pool.tile([128, N], bf16, tag="ybf", name="y_bf")[:, :ncols]
            for j in range(ncols // PCH):
                ps = proj_psum.tile([128, PCH], f32, tag="proj", name="proj_ps")
                sl = slice(j * PCH, (j + 1) * PCH)
                nc.tensor.matmul(
                    ps[:], wqkT_sb[0][:], x_lo[:, sl], start=True, stop=False
                )
                nc.tensor.matmul(
                    ps[:], wqkT_sb[1][:], x_hi[:, sl], start=False, stop=True
                )
                nc.any.tensor_copy(y_bf[:, sl], ps[:])

            ysq = ysq_pool.tile([128, N], bf16, tag="ysq", name="ysq")[:, :ncols]
            nc.vector.tensor_mul(ysq, y_bf, y_bf)

            lnn2 = lnn2_pool.tile([128, N], f32, tag="lnn2", name="lnn2")[:, :ncols]
            for j in range(ncols // NCH):
                ps = n2_psum.tile([128, NCH], f32, tag="n2", name="n2_ps")
                sl = slice(j * NCH, (j + 1) * NCH)
                for c in range(NCH // PCH):
                    csl = slice(j * NCH + c * PCH, j * NCH + (c + 1) * PCH)
                    nc.tensor.matmul(
                        ps[:, c * PCH : (c + 1) * PCH],
                        ones_sb[:],
                        ysq[:, csl],
                        start=True,
                        stop=True,
                    )
                nc.scalar.activation(out=lnn2[:, sl], in_=ps[:], func=AF.Ln)

            # rk = exp(-0.5 * ln(n2)) = n2^-0.5, broadcast across partitions already
            rk = rk_pool.tile([128, N], f32, tag="rk", name="rk")[:, :ncols]
            nc.scalar.activation(out=rk, in_=lnn2, func=AF.Exp, scale=-0.5)

            xn = kn_pool.tile([128, ncols], bf16, tag=tagbase, name=tagbase)
            nc.vector.tensor_mul(xn[:], y_bf, rk)
            return xn

        qn = phase_a(xq_ext, QBLK, "qn")
        k1n = phase_a(f1_ext, N, "k1n")
        k2n = phase_a(f2_ext, N, "k2n")

        def phase_b(kn, s):
            """row block of sim + softmax for one K map, streamed to out[s]."""
            for t in range(QBLK // 128):
                lhsT = qn[:, t * 128 : (t + 1) * 128]
                attn = attn_pool.tile([128, N], bf16, tag="attn", name="attn")
                stile = stat_pool.tile([128, 4], f32, tag="stile", name="stile")
                e_chunks = []
                for j in range(N // NCH):
                    ps = sim_psum.tile([128, NCH], f32, tag="sim", name="sim_ps")
                    for c in range(NCH // PCH):
                        csl = slice(j * NCH + c * PCH, j * NCH + (c + 1) * PCH)
                        nc.tensor.matmul(
                            ps[:, c * PCH : (c + 1) * PCH],
                            lhsT,
                            kn[:, csl],
                            start=True,
                            stop=True,
                        )
                    e = e_pool.tile([128, NCH], bf16, tag="e", name="e")
                    nc.scalar.activation(
                        out=e[:],
                        in_=ps[:],
                        func=AF.Exp,
                        accum_out=stile[:, j : j + 1],
                    )
                    e_chunks.append(e)
                ssum = stat_pool.tile([128, 1], f32, tag="ssum", name="ssum")
                nc.vector.reduce_sum(ssum[:], stile[:], axis=mybir.AxisListType.X)
                recip = stat_pool.tile([128, 1], f32, tag="recip", name="recip")
                nc.vector.reciprocal(recip[:], ssum[:])
                for j, e in enumerate(e_chunks):
                    nc.vector.tensor_scalar_mul(
                        attn[:, j * NCH : (j + 1) * NCH], e[:], recip[:]
                    )
                nc.sync.dma_start(
                    out=out_ext[s, t * 128 : (t + 1) * 128, :], in_=attn[:]
                )

        phase_b(k1n, 0)
        phase_b(k2n, 1)

    nc.compile()
    return nc


def _get_nc():
    if "nc" not in _cached:
        _cached["nc"] = _build()
    return _cached["nc"]


def kernel(fmap1, fmap2, dmap, Wqk):
    from concourse.bass_utils import run_bass_kernel_spmd

    bf = ml_dtypes.bfloat16
    f1r = np.asarray(fmap1, dtype=np.float32).reshape(B, C, N)
    f2r = np.asarray(fmap2, dtype=np.float32).reshape(B, C, N)
    dqr = np.asarray(dmap, dtype=np.float32).reshape(B, C, N)
    wT = np.ascontiguousarray(np.asarray(Wqk, dtype=np.float32).T).astype(bf)

    in_maps = []
    for i in range(N_CORES):
        b, r = divmod(i, 4)
        in_maps.append(
            {
                "f1": np.ascontiguousarray(f1r[b]).astype(bf),
                "f2": np.ascontiguousarray(f2r[b]).astype(bf),
                "xq": np.ascontiguousarray(
                    dqr[b][:, r * QBLK : (r + 1) * QBLK]
                ).astype(bf),
                "wqkT": wT,
            }
        )

    nc = _get_nc()
    res = run_bass_kernel_spmd(nc, in_maps, core_ids=list(range(N_CORES)))
    _cached["last_result"] = res

    attn1 = np.empty((B, 1, N, N), dtype=np.float32)
    attn2 = np.empty((B, 1, N, N), dtype=np.float32)
    for i in range(N_CORES):
        b, r = divmod(i, 4)
        o = res.results[i]["out"]
        attn1[b, 0, r * QBLK : (r + 1) * QBLK, :] = o[0].astype(np.float32)
        attn2[b, 0, r * QBLK : (r + 1) * QBLK, :] = o[1].astype(np.float32)
    return (attn1, attn2)


# revision 4
# speedup vs baseline: 1.1447x; 1.1447x over previous
"""Trainium2 Bass kernel for nn_Attention_28930899706081 (sparse_attention).

Reference computation:
  k1 = l2norm_c(Wqk @ fmap1), k2 = l2norm_c(Wqk @ fmap2), q = l2norm_c(Wqk @ dmap)
  sim_i = q^T k_i per batch  -> [b, n, n] with n = h*w = 4096
  attn_i = softmax(sim_i, axis=-1)[:, None]  -> [b, 1, n, n]
  returns (attn1, attn2)

Sharding: 8 cores; core i handles batch b = i//4 and query-row block r = i%4
(1024 of 4096 rows). Each core computes the full normalized K for its batch
(recompute instead of collectives) and its row block of both sims + softmax.

Compute dtype bf16 (fp32 accumulation in PSUM); |sim| <= 1 because q/k are
unit vectors, so softmax needs no max subtraction. Row sums come from the
ScalarE activation accumulator fused with exp. Output is written bf16 and
upcast to f32 on the host.
"""

import numpy as np
import ml_dtypes

B, C, H, W, D = 2, 256, 64, 64, 128
N = H * W  # 4096
QBLK = N // 4  # 1024 query rows per core
N_CORES = 8

_cached = {}


def _build():
    import concourse.mybir as mybir
    import concourse.tile as tile
    from concourse import bacc
    from contextlib import ExitStack

    f32 = mybir.dt.float32
    bf16 = mybir.dt.bfloat16
    AF = mybir.ActivationFunctionType

    nc = bacc.Bacc(
        "TRN2",
        target_bir_lowering=False,
        debug=False,
        enable_asserts=True,
        num_devices=N_CORES,
    )

    f1_ext = nc.dram_tensor("f1", [C, N], bf16, kind="ExternalInput").ap()
    f2_ext = nc.dram_tensor("f2", [C, N], bf16, kind="ExternalInput").ap()
    xq_ext = nc.dram_tensor("xq", [C, QBLK], bf16, kind="ExternalInput").ap()
    wqkT_ext = nc.dram_tensor("wqkT", [C, D], bf16, kind="ExternalInput").ap()
    out_ext = nc.dram_tensor("out", [2, QBLK, N], bf16, kind="ExternalOutput").ap()

    PCH = 512  # matmul free-dim chunk (one PSUM bank)
    ECH = 2048  # exp / sim-psum chunk (4 banks)

    with tile.TileContext(nc) as tc, ExitStack() as ctx:
        consts = ctx.enter_context(tc.tile_pool(name="consts", bufs=1))
        xin = ctx.enter_context(tc.tile_pool(name="xin", bufs=4))
        ybf_pool = ctx.enter_context(tc.tile_pool(name="ybf", bufs=2))
        ysq_pool = ctx.enter_context(tc.tile_pool(name="ysq", bufs=2))
        lnn2_pool = ctx.enter_context(tc.tile_pool(name="lnn2", bufs=2))
        rk_pool = ctx.enter_context(tc.tile_pool(name="rk", bufs=2))
        kn_pool = ctx.enter_context(tc.tile_pool(name="kn", bufs=1))
        e_pool = ctx.enter_context(tc.tile_pool(name="epool", bufs=4))
        attn_pool = ctx.enter_context(tc.tile_pool(name="attn", bufs=2))
        stat_pool = ctx.enter_context(tc.tile_pool(name="stat", bufs=4))

        # constants
        wqkT_sb = [
            consts.tile([128, D], bf16, tag=f"wqkT{k}", name=f"wqkT{k}")
            for k in range(2)
        ]
        nc.sync.dma_start(out=wqkT_sb[0][:], in_=wqkT_ext[0:128, :])
        nc.sync.dma_start(out=wqkT_sb[1][:], in_=wqkT_ext[128:256, :])
        ones_sb = consts.tile([128, 128], bf16, tag="ones", name="ones")
        nc.vector.memset(ones_sb[:], 1.0)

        with tc.tile_pool(name="proj_psum", bufs=2, space="PSUM") as proj_psum, \
             tc.tile_pool(name="n2_psum", bufs=1, space="PSUM") as n2_psum:

            def phase_a(x_ext, ncols, tagbase):
                """project + l2-normalize columns -> [128, ncols] bf16."""
                x_lo = xin.tile([128, N], bf16, tag="xin", name="x_lo")[:, :ncols]
                x_hi = xin.tile([128, N], bf16, tag="xin", name="x_hi")[:, :ncols]
                nc.sync.dma_start(out=x_lo, in_=x_ext[0:128, :])
                nc.sync.dma_start(out=x_hi, in_=x_ext[128:256, :])

                y_bf = ybf_pool.tile([128, N], bf16, tag="ybf", name="y_bf")[:, :ncols]
                for j in range(ncols // 1024):
                    ps = proj_psum.tile([128, 1024], f32, tag="proj", name="proj_ps")
                    for c in range(2):
                        sl = slice(j * 1024 + c * PCH, j * 1024 + (c + 1) * PCH)
                        psl = ps[:, c * PCH : (c + 1) * PCH]
                        nc.tensor.matmul(
                            psl, wqkT_sb[0][:], x_lo[:, sl], start=True, stop=False
                        )
                        nc.tensor.matmul(
                            psl, wqkT_sb[1][:], x_hi[:, sl], start=False, stop=True
                        )
                    nc.vector.tensor_copy(y_bf[:, j * 1024 : (j + 1) * 1024], ps[:])

                ysq = ysq_pool.tile([128, N], bf16, tag="ysq", name="ysq")[:, :ncols]
                nc.vector.tensor_mul(ysq, y_bf, y_bf)

                lnn2 = lnn2_pool.tile([128, N], f32, tag="lnn2", name="lnn2")[
                    :, :ncols
                ]
                for j in range(max(1, ncols // ECH)):
                    cw = min(ECH, ncols)
                    ps = n2_psum.tile([128, ECH], f32, tag="n2", name="n2_ps")[:, :cw]
                    for c in range(cw // PCH):
                        csl = slice(j * ECH + c * PCH, j * ECH + (c + 1) * PCH)
                        nc.tensor.matmul(
                            ps[:, c * PCH : (c + 1) * PCH],
                            ones_sb[:],
                            ysq[:, csl],
                            start=True,
                            stop=True,
                        )
                    nc.scalar.activation(
                        out=lnn2[:, j * ECH : j * ECH + cw], in_=ps[:], func=AF.Ln
                    )

                # rk = exp(-0.5 * ln(n2)) = n2^-0.5 (already partition-broadcast)
                rk = rk_pool.tile([128, N], f32, tag="rk", name="rk")[:, :ncols]
                nc.scalar.activation(out=rk, in_=lnn2, func=AF.Exp, scale=-0.5)

                xn = kn_pool.tile([128, ncols], bf16, tag=tagbase, name=tagbase)
                nc.vector.tensor_mul(xn[:], y_bf, rk)
                return xn

            qn = phase_a(xq_ext, QBLK, "qn")
            k1n = phase_a(f1_ext, N, "k1n")
            k2n = phase_a(f2_ext, N, "k2n")

        with tc.tile_pool(name="sim_psum", bufs=2, space="PSUM") as sim_psum:

            def phase_b(kn, s):
                """row block of sim + softmax for one K map, streamed to out[s]."""
                for t in range(QBLK // 128):
                    lhsT = qn[:, t * 128 : (t + 1) * 128]
                    attn = attn_pool.tile([128, N], bf16, tag="attn", name="attn")
                    stile = stat_pool.tile([128, 2], f32, tag="stile", name="stile")
                    e_chunks = []
                    for j in range(N // ECH):
                        ps = sim_psum.tile([128, ECH], f32, tag="sim", name="sim_ps")
                        for c in range(ECH // PCH):
                            csl = slice(j * ECH + c * PCH, j * ECH + (c + 1) * PCH)
                            nc.tensor.matmul(
                                ps[:, c * PCH : (c + 1) * PCH],
                                lhsT,
                                kn[:, csl],
                                start=True,
                                stop=True,
                            )
                        e = e_pool.tile([128, ECH], bf16, tag="e", name="e")
                        nc.scalar.activation(
                            out=e[:],
                            in_=ps[:],
                            func=AF.Exp,
                            accum_out=stile[:, j : j + 1],
                        )
                        e_chunks.append(e)
                    ssum = stat_pool.tile([128, 1], f32, tag="ssum", name="ssum")
                    nc.vector.reduce_sum(ssum[:], stile[:], axis=mybir.AxisListType.X)
                    recip = stat_pool.tile([128, 1], f32, tag="recip", name="recip")
                    nc.vector.reciprocal(recip[:], ssum[:])
                    for j, e in enumerate(e_chunks):
                        nc.vector.tensor_scalar_mul(
                            attn[:, j * ECH : (j + 1) * ECH], e[:], recip[:]
                        )
                    nc.sync.dma_start(
                        out=out_ext[s, t * 128 : (t + 1) * 128, :], in_=attn[:]
                    )

            phase_b(k1n, 0)
            phase_b(k2n, 1)

    nc.compile()
    return nc


def _get_nc():
    if "nc" not in _cached:
        _cached["nc"] = _build()
    return _cached["nc"]


def _in_maps(fmap1, fmap2, dmap, Wqk):
    bf = ml_dtypes.bfloat16
    f1r = np.asarray(fmap1, dtype=np.float32).reshape(B, C, N)
    f2r = np.asarray(fmap2, dtype=np.float32).reshape(B, C, N)
    dqr = np.asarray(dmap, dtype=np.float32).reshape(B, C, N)
    wT = np.ascontiguousarray(np.asarray(Wqk, dtype=np.float32).T).astype(bf)

    in_maps = []
    for i in range(N_CORES):
        b, r = divmod(i, 4)
        in_maps.append(
            {
                "f1": np.ascontiguousarray(f1r[b]).astype(bf),
                "f2": np.ascontiguousarray(f2r[b]).astype(bf),
                "xq": np.ascontiguousarray(
                    dqr[b][:, r * QBLK : (r + 1) * QBLK]
                ).astype(bf),
                "wqkT": wT,
            }
        )
    return in_maps


def kernel(fmap1, fmap2, dmap, Wqk):
    from concourse.bass_utils import run_bass_kernel_spmd

    in_maps = _in_maps(fmap1, fmap2, dmap, Wqk)
    nc = _get_nc()
    res = run_bass_kernel_spmd(nc, in_maps, core_ids=list(range(N_CORES)))
    _cached["last_result"] = res

    attn1 = np.empty((B, 1, N, N), dtype=np.float32)
    attn2 = np.empty((B, 1, N, N), dtype=np.float32)
    for i in range(N_CORES):
        b, r = divmod(i, 4)
        o = res.results[i]["out"]
        attn1[b, 0, r * QBLK : (r + 1) * QBLK, :] = o[0].astype(np.float32)
        attn2[b, 0, r * QBLK : (r + 1) * QBLK, :] = o[1].astype(np.float32)
    return (attn1, attn2)


# revision 5
# speedup vs baseline: 1.1882x; 1.0380x over previous
"""Trainium2 Bass kernel for nn_Attention_28930899706081 (sparse_attention).

Reference computation:
  k1 = l2norm_c(Wqk @ fmap1), k2 = l2norm_c(Wqk @ fmap2), q = l2norm_c(Wqk @ dmap)
  sim_i = q^T k_i per batch  -> [b, n, n] with n = h*w = 4096
  attn_i = softmax(sim_i, axis=-1)[:, None]  -> [b, 1, n, n]
  returns (attn1, attn2)

Sharding: 8 cores; core i handles batch b = i//4 and query-row block r = i%4
(1024 of 4096 rows). Each core computes the full normalized K for its batch
(recompute instead of collectives) and its row block of both sims + softmax.

Compute dtype bf16 (fp32 accumulation in PSUM); |sim| <= 1 because q/k are
unit vectors, so softmax needs no max subtraction. Row sums come from the
ScalarE activation accumulator fused with exp. rsqrt for the L2 norm is
exp(-0.5*ln(x)) — Ln/Exp stay in one ACT table-set family, and all Lns are
batched before all Exps (ordering edges) so the table loads happen twice
total instead of thrashing. Output is written bf16 and upcast on the host.
"""

import numpy as np
import ml_dtypes

B, C, H, W, D = 2, 256, 64, 64, 128
N = H * W  # 4096
QBLK = N // 4  # 1024 query rows per core
N_CORES = 8

_cached = {}


def _build():
    import concourse.mybir as mybir
    import concourse.tile as tile
    from concourse.tile_rust import add_dep_helper
    from concourse import bacc
    from contextlib import ExitStack

    f32 = mybir.dt.float32
    bf16 = mybir.dt.bfloat16
    AF = mybir.ActivationFunctionType

    nc = bacc.Bacc(
        "TRN2",
        target_bir_lowering=False,
        debug=False,
        enable_asserts=True,
        num_devices=N_CORES,
    )

    f1_ext = nc.dram_tensor("f1", [C, N], bf16, kind="ExternalInput").ap()
    f2_ext = nc.dram_tensor("f2", [C, N], bf16, kind="ExternalInput").ap()
    xq_ext = nc.dram_tensor("xq", [C, QBLK], bf16, kind="ExternalInput").ap()
    wqkT_ext = nc.dram_tensor("wqkT", [C, D], bf16, kind="ExternalInput").ap()
    out_ext = nc.dram_tensor("out", [2, QBLK, N], bf16, kind="ExternalOutput").ap()

    PCH = 512  # matmul free-dim chunk (one PSUM bank)
    CH = 2048  # pipeline chunk

    with tile.TileContext(nc) as tc, ExitStack() as ctx:
        consts = ctx.enter_context(tc.tile_pool(name="consts", bufs=1))
        xin = ctx.enter_context(tc.tile_pool(name="xin", bufs=6))
        ybf_pool = ctx.enter_context(tc.tile_pool(name="ybf", bufs=3))
        ysq_pool = ctx.enter_context(tc.tile_pool(name="ysq", bufs=2))
        lnn2_pool = ctx.enter_context(tc.tile_pool(name="lnn2", bufs=3))
        rk_pool = ctx.enter_context(tc.tile_pool(name="rk", bufs=3))
        kn_pool = ctx.enter_context(tc.tile_pool(name="kn", bufs=1))
        e_pool = ctx.enter_context(tc.tile_pool(name="epool", bufs=4))
        attn_pool = ctx.enter_context(tc.tile_pool(name="attn", bufs=2))
        stat_pool = ctx.enter_context(tc.tile_pool(name="stat", bufs=4))

        # constants
        wqkT_sb = [
            consts.tile([128, D], bf16, tag=f"wqkT{k}", name=f"wqkT{k}")
            for k in range(2)
        ]
        nc.sync.dma_start(out=wqkT_sb[0][:], in_=wqkT_ext[0:128, :])
        nc.sync.dma_start(out=wqkT_sb[1][:], in_=wqkT_ext[128:256, :])
        ones_sb = consts.tile([128, 128], bf16, tag="ones", name="ones")
        nc.vector.memset(ones_sb[:], 1.0)

        maps = []  # per map: dict with y_bf, lnn2, ncols, tag

        with tc.tile_pool(name="proj_psum", bufs=2, space="PSUM") as proj_psum, \
             tc.tile_pool(name="n2_psum", bufs=1, space="PSUM") as n2_psum:

            def phase_a1(x_ext, ncols, tagbase):
                """DMA + project + square + col-norm + ln, chunk-pipelined."""
                y_bf = ybf_pool.tile([128, N], bf16, tag="ybf", name="y_bf")[:, :ncols]
                lnn2 = lnn2_pool.tile([128, N], f32, tag="lnn2", name="lnn2")[
                    :, :ncols
                ]
                last_ln = None
                for j in range(max(1, ncols // CH)):
                    cw = min(CH, ncols)
                    j0 = j * CH
                    x_lo = xin.tile([128, CH], bf16, tag="xin", name="x_lo")[:, :cw]
                    x_hi = xin.tile([128, CH], bf16, tag="xin", name="x_hi")[:, :cw]
                    nc.sync.dma_start(out=x_lo, in_=x_ext[0:128, j0 : j0 + cw])
                    nc.sync.dma_start(out=x_hi, in_=x_ext[128:256, j0 : j0 + cw])

                    for h in range(cw // 1024):
                        ps = proj_psum.tile([128, 1024], f32, tag="proj", name="pps")
                        for c in range(2):
                            sl = slice(h * 1024 + c * PCH, h * 1024 + (c + 1) * PCH)
                            psl = ps[:, c * PCH : (c + 1) * PCH]
                            nc.tensor.matmul(
                                psl, wqkT_sb[0][:], x_lo[:, sl], start=True, stop=False
                            )
                            nc.tensor.matmul(
                                psl, wqkT_sb[1][:], x_hi[:, sl], start=False, stop=True
                            )
                        nc.vector.tensor_copy(
                            y_bf[:, j0 + h * 1024 : j0 + (h + 1) * 1024], ps[:]
                        )

                    ysq = ysq_pool.tile([128, CH], bf16, tag="ysq", name="ysq")[:, :cw]
                    nc.vector.tensor_mul(ysq, y_bf[:, j0 : j0 + cw], y_bf[:, j0 : j0 + cw])

                    nps = n2_psum.tile([128, CH], f32, tag="n2", name="nps")[:, :cw]
                    for c in range(cw // PCH):
                        nc.tensor.matmul(
                            nps[:, c * PCH : (c + 1) * PCH],
                            ones_sb[:],
                            ysq[:, c * PCH : (c + 1) * PCH],
                            start=True,
                            stop=True,
                        )
                    ln = nc.scalar.activation(
                        out=lnn2[:, j0 : j0 + cw], in_=nps, func=AF.Ln
                    )
                    last_ln = ln
                maps.append(
                    dict(y_bf=y_bf, lnn2=lnn2, ncols=ncols, tag=tagbase)
                )
                return last_ln

            phase_a1(xq_ext, QBLK, "qn")
            phase_a1(f1_ext, N, "k1n")
            last_ln = phase_a1(f2_ext, N, "k2n")

            # batched: rk = exp(-0.5*ln(n2)); xn = y * rk   (chunked)
            norm_tiles = {}
            first_exp = None
            for m in maps:
                ncols, y_bf, lnn2 = m["ncols"], m["y_bf"], m["lnn2"]
                xn = kn_pool.tile([128, ncols], bf16, tag=m["tag"], name=m["tag"])
                for j in range(max(1, ncols // CH)):
                    cw = min(CH, ncols)
                    j0 = j * CH
                    rk = rk_pool.tile([128, CH], f32, tag="rk", name="rk")[:, :cw]
                    ex = nc.scalar.activation(
                        out=rk, in_=lnn2[:, j0 : j0 + cw], func=AF.Exp, scale=-0.5
                    )
                    if first_exp is None:
                        first_exp = ex
                        add_dep_helper(
                            ex.ins, last_ln.ins, sync=False,
                            reason="batch all Lns before first Exp (table set)",
                        )
                    nc.vector.tensor_mul(
                        xn[:, j0 : j0 + cw], y_bf[:, j0 : j0 + cw], rk
                    )
                norm_tiles[m["tag"]] = xn

            qn = norm_tiles["qn"]
            k1n = norm_tiles["k1n"]
            k2n = norm_tiles["k2n"]

        with tc.tile_pool(name="sim_psum", bufs=2, space="PSUM") as sim_psum:

            def phase_b(kn, s):
                """row block of sim + softmax for one K map, streamed to out[s]."""
                for t in range(QBLK // 128):
                    lhsT = qn[:, t * 128 : (t + 1) * 128]
                    attn = attn_pool.tile([128, N], bf16, tag="attn", name="attn")
                    stile = stat_pool.tile([128, 2], f32, tag="stile", name="stile")
                    e_chunks = []
                    for j in range(N // CH):
                        ps = sim_psum.tile([128, CH], f32, tag="sim", name="sim_ps")
                        for c in range(CH // PCH):
                            csl = slice(j * CH + c * PCH, j * CH + (c + 1) * PCH)
                            nc.tensor.matmul(
                                ps[:, c * PCH : (c + 1) * PCH],
                                lhsT,
                                kn[:, csl],
                                start=True,
                                stop=True,
                            )
                        e = e_pool.tile([128, CH], bf16, tag="e", name="e")
                        nc.scalar.activation(
                            out=e[:],
                            in_=ps[:],
                            func=AF.Exp,
                            accum_out=stile[:, j : j + 1],
                        )
                        e_chunks.append(e)
                    ssum = stat_pool.tile([128, 1], f32, tag="ssum", name="ssum")
                    nc.vector.reduce_sum(ssum[:], stile[:], axis=mybir.AxisListType.X)
                    recip = stat_pool.tile([128, 1], f32, tag="recip", name="recip")
                    nc.vector.reciprocal(recip[:], ssum[:])
                    for j, e in enumerate(e_chunks):
                        nc.vector.tensor_scalar_mul(
                            attn[:, j * CH : (j + 1) * CH], e[:], recip[:]
                        )
                    nc.sync.dma_start(
                        out=out_ext[s, t * 128 : (t + 1) * 128, :], in_=attn[:]
                    )

            phase_b(k1n, 0)
            phase_b(k2n, 1)

    nc.compile()
    return nc


def _get_nc():
    if "nc" not in _cached:
        _cached["nc"] = _build()
    return _cached["nc"]


def _in_maps(fmap1, fmap2, dmap, Wqk):
    bf = ml_dtypes.bfloat16
    f1r = np.asarray(fmap1, dtype=np.float32).reshape(B, C, N)
    f2r = np.asarray(fmap2, dtype=np.float32).reshape(B, C, N)
    dqr = np.asarray(dmap, dtype=np.float32).reshape(B, C, N)
    wT = np.ascontiguousarray(np.asarray(Wqk, dtype=np.float32).T).astype(bf)

    in_maps = []
    for i in range(N_CORES):
        b, r = divmod(i, 4)
        in_maps.append(
            {
                "f1": np.ascontiguousarray(f1r[b]).astype(bf),
                "f2": np.ascontiguousarray(f2r[b]).astype(bf),
                "xq": np.ascontiguousarray(
                    dqr[b][:, r * QBLK : (r + 1) * QBLK]
                ).astype(bf),
                "wqkT": wT,
            }
        )
    return in_maps


def kernel(fmap1, fmap2, dmap, Wqk):
    from concourse.bass_utils import run_bass_kernel_spmd

    in_maps = _in_maps(fmap1, fmap2, dmap, Wqk)
    nc = _get_nc()
    res = run_bass_kernel_spmd(nc, in_maps, core_ids=list(range(N_CORES)))
    _cached["last_result"] = res

    attn1 = np.empty((B, 1, N, N), dtype=np.float32)
    attn2 = np.empty((B, 1, N, N), dtype=np.float32)
    for i in range(N_CORES):
        b, r = divmod(i, 4)
        o = res.results[i]["out"]
        attn1[b, 0, r * QBLK : (r + 1) * QBLK, :] = o[0].astype(np.float32)
        attn2[b, 0, r * QBLK : (r + 1) * QBLK, :] = o[1].astype(np.float32)
    return (attn1, attn2)


# revision 6
# speedup vs baseline: 1.2202x; 1.0270x over previous
"""Trainium2 Bass kernel for nn_Attention_28930899706081 (sparse_attention).

Reference computation:
  k1 = l2norm_c(Wqk @ fmap1), k2 = l2norm_c(Wqk @ fmap2), q = l2norm_c(Wqk @ dmap)
  sim_i = q^T k_i per batch  -> [b, n, n] with n = h*w = 4096
  attn_i = softmax(sim_i, axis=-1)[:, None]  -> [b, 1, n, n]
  returns (attn1, attn2)

Sharding: 8 cores; core i handles batch b = i//4 and query-row block r = i%4
(1024 of 4096 rows). Each core computes the full normalized K for its batch
(recompute instead of collectives) and its row block of both sims + softmax.

Compute dtype bf16 (fp32 accumulation in PSUM); |sim| <= 1 because q/k are
unit vectors, so softmax needs no max subtraction. Row sums come from the
ScalarE activation accumulator fused with exp. Column L2 norms are computed
with a ones-matmul partition reduction (broadcast across partitions), and
1/sqrt comes from the single-pass Abs_reciprocal_sqrt activation (measured
~4e-5 rel err on HW). Output is written bf16 and upcast on the host.
"""

import numpy as np
import ml_dtypes

B, C, H, W, D = 2, 256, 64, 64, 128
N = H * W  # 4096
QBLK = N // 4  # 1024 query rows per core
N_CORES = 8

_cached = {}


def _build():
    import concourse.mybir as mybir
    import concourse.tile as tile
    from concourse.tile_rust import add_dep_helper
    from concourse import bacc
    from contextlib import ExitStack

    f32 = mybir.dt.float32
    bf16 = mybir.dt.bfloat16
    AF = mybir.ActivationFunctionType

    nc = bacc.Bacc(
        "TRN2",
        target_bir_lowering=False,
        debug=False,
        enable_asserts=True,
        num_devices=N_CORES,
    )

    f1_ext = nc.dram_tensor("f1", [C, N], bf16, kind="ExternalInput").ap()
    f2_ext = nc.dram_tensor("f2", [C, N], bf16, kind="ExternalInput").ap()
    xq_ext = nc.dram_tensor("xq", [C, QBLK], bf16, kind="ExternalInput").ap()
    wqkT_ext = nc.dram_tensor("wqkT", [C, D], bf16, kind="ExternalInput").ap()
    out_ext = nc.dram_tensor("out", [2, QBLK, N], bf16, kind="ExternalOutput").ap()

    PCH = 512  # matmul free-dim chunk (one PSUM bank)
    CH = 2048  # pipeline chunk

    with tile.TileContext(nc) as tc, ExitStack() as ctx:
        consts = ctx.enter_context(tc.tile_pool(name="consts", bufs=1))
        xin = ctx.enter_context(tc.tile_pool(name="xin", bufs=8))
        ybf_pool = ctx.enter_context(tc.tile_pool(name="ybf", bufs=3))
        ysq_pool = ctx.enter_context(tc.tile_pool(name="ysq", bufs=2))
        rk_pool = ctx.enter_context(tc.tile_pool(name="rk", bufs=3))
        kn_pool = ctx.enter_context(tc.tile_pool(name="kn", bufs=1))
        e_pool = ctx.enter_context(tc.tile_pool(name="epool", bufs=4))
        attn_pool = ctx.enter_context(tc.tile_pool(name="attn", bufs=2))
        stat_pool = ctx.enter_context(tc.tile_pool(name="stat", bufs=4))

        # constants
        wqkT_sb = [
            consts.tile([128, D], bf16, tag=f"wqkT{k}", name=f"wqkT{k}")
            for k in range(2)
        ]
        nc.sync.dma_start(out=wqkT_sb[0][:], in_=wqkT_ext[0:128, :])
        nc.sync.dma_start(out=wqkT_sb[1][:], in_=wqkT_ext[128:256, :])
        ones_sb = consts.tile([128, 128], bf16, tag="ones", name="ones")
        nc.vector.memset(ones_sb[:], 1.0)

        last_rk = None

        with tc.tile_pool(name="proj_psum", bufs=2, space="PSUM") as proj_psum, \
             tc.tile_pool(name="n2_psum", bufs=1, space="PSUM") as n2_psum:

            def phase_a(x_ext, ncols, tagbase):
                """DMA + project + l2-normalize columns, chunk-pipelined."""
                nonlocal last_rk
                y_bf = ybf_pool.tile([128, N], bf16, tag="ybf", name="y_bf")[:, :ncols]
                xn = kn_pool.tile([128, ncols], bf16, tag=tagbase, name=tagbase)
                for j in range(max(1, ncols // CH)):
                    cw = min(CH, ncols)
                    j0 = j * CH
                    x_lo = xin.tile([128, CH], bf16, tag="xin", name="x_lo")[:, :cw]
                    x_hi = xin.tile([128, CH], bf16, tag="xin", name="x_hi")[:, :cw]
                    nc.sync.dma_start(out=x_lo, in_=x_ext[0:128, j0 : j0 + cw])
                    nc.sync.dma_start(out=x_hi, in_=x_ext[128:256, j0 : j0 + cw])

                    for h in range(cw // 1024):
                        ps = proj_psum.tile([128, 1024], f32, tag="proj", name="pps")
                        for c in range(2):
                            sl = slice(h * 1024 + c * PCH, h * 1024 + (c + 1) * PCH)
                            psl = ps[:, c * PCH : (c + 1) * PCH]
                            nc.tensor.matmul(
                                psl, wqkT_sb[0][:], x_lo[:, sl], start=True, stop=False
                            )
                            nc.tensor.matmul(
                                psl, wqkT_sb[1][:], x_hi[:, sl], start=False, stop=True
                            )
                        nc.any.tensor_copy(
                            y_bf[:, j0 + h * 1024 : j0 + (h + 1) * 1024], ps[:]
                        )

                    ysq = ysq_pool.tile([128, CH], bf16, tag="ysq", name="ysq")[:, :cw]
                    ysl = y_bf[:, j0 : j0 + cw]
                    nc.vector.tensor_mul(ysq, ysl, ysl)

                    nps = n2_psum.tile([128, CH], f32, tag="n2", name="nps")[:, :cw]
                    for c in range(cw // PCH):
                        nc.tensor.matmul(
                            nps[:, c * PCH : (c + 1) * PCH],
                            ones_sb[:],
                            ysq[:, c * PCH : (c + 1) * PCH],
                            start=True,
                            stop=True,
                        )
                    # rk = n2^-0.5, already broadcast across partitions
                    rk = rk_pool.tile([128, CH], f32, tag="rk", name="rk")[:, :cw]
                    last_rk = nc.scalar.activation(
                        out=rk, in_=nps, func=AF.Abs_reciprocal_sqrt
                    )
                    nc.vector.tensor_mul(xn[:, j0 : j0 + cw], ysl, rk)
                return xn

            qn = phase_a(xq_ext, QBLK, "qn")
            k1n = phase_a(f1_ext, N, "k1n")
            k2n = phase_a(f2_ext, N, "k2n")

        with tc.tile_pool(name="sim_psum", bufs=2, space="PSUM") as sim_psum:
            first_exp = None

            def phase_b(kn, s):
                """row block of sim + softmax for one K map, streamed to out[s]."""
                nonlocal first_exp
                for t in range(QBLK // 128):
                    lhsT = qn[:, t * 128 : (t + 1) * 128]
                    attn = attn_pool.tile([128, N], bf16, tag="attn", name="attn")
                    stile = stat_pool.tile([128, 2], f32, tag="stile", name="stile")
                    e_chunks = []
                    for j in range(N // CH):
                        ps = sim_psum.tile([128, CH], f32, tag="sim", name="sim_ps")
                        for c in range(CH // PCH):
                            csl = slice(j * CH + c * PCH, j * CH + (c + 1) * PCH)
                            nc.tensor.matmul(
                                ps[:, c * PCH : (c + 1) * PCH],
                                lhsT,
                                kn[:, csl],
                                start=True,
                                stop=True,
                            )
                        e = e_pool.tile([128, CH], bf16, tag="e", name="e")
                        ex = nc.scalar.activation(
                            out=e[:],
                            in_=ps[:],
                            func=AF.Exp,
                            accum_out=stile[:, j : j + 1],
                        )
                        if first_exp is None:
                            first_exp = ex
                            # keep ACT table loads to 2: all Abs_reciprocal_sqrt
                            # (phase A) strictly before any Exp (phase B)
                            add_dep_helper(
                                ex.ins, last_rk.ins, sync=False,
                                reason="order rk (ars table) before exp table load",
                            )
                        e_chunks.append(e)
                    ssum = stat_pool.tile([128, 1], f32, tag="ssum", name="ssum")
                    nc.vector.reduce_sum(ssum[:], stile[:], axis=mybir.AxisListType.X)
                    recip = stat_pool.tile([128, 1], f32, tag="recip", name="recip")
                    nc.vector.reciprocal(recip[:], ssum[:])
                    for j, e in enumerate(e_chunks):
                        nc.vector.tensor_scalar_mul(
                            attn[:, j * CH : (j + 1) * CH], e[:], recip[:]
                        )
                    nc.sync.dma_start(
                        out=out_ext[s, t * 128 : (t + 1) * 128, :], in_=attn[:]
                    )

            phase_b(k1n, 0)
            phase_b(k2n, 1)

    nc.compile()
    return nc


def _get_nc():
    if "nc" not in _cached:
        _cached["nc"] = _build()
    return _cached["nc"]


def _in_maps(fmap1, fmap2, dmap, Wqk):
    bf = ml_dtypes.bfloat16
    f1r = np.asarray(fmap1, dtype=np.float32).reshape(B, C, N)
    f2r = np.asarray(fmap2, dtype=np.float32).reshape(B, C, N)
    dqr = np.asarray(dmap, dtype=np.float32).reshape(B, C, N)
    wT = np.ascontiguousarray(np.asarray(Wqk, dtype=np.float32).T).astype(bf)

    in_maps = []
    for i in range(N_CORES):
        b, r = divmod(i, 4)
        in_maps.append(
            {
                "f1": np.ascontiguousarray(f1r[b]).astype(bf),
                "f2": np.ascontiguousarray(f2r[b]).astype(bf),
                "xq": np.ascontiguousarray(
                    dqr[b][:, r * QBLK : (r + 1) * QBLK]
                ).astype(bf),
                "wqkT": wT,
            }
        )
    return in_maps


def kernel(fmap1, fmap2, dmap, Wqk):
    from concourse.bass_utils import run_bass_kernel_spmd

    in_maps = _in_maps(fmap1, fmap2, dmap, Wqk)
    nc = _get_nc()
    res = run_bass_kernel_spmd(nc, in_maps, core_ids=list(range(N_CORES)))
    _cached["last_result"] = res

    attn1 = np.empty((B, 1, N, N), dtype=np.float32)
    attn2 = np.empty((B, 1, N, N), dtype=np.float32)
    for i in range(N_CORES):
        b, r = divmod(i, 4)
        o = res.results[i]["out"]
        attn1[b, 0, r * QBLK : (r + 1) * QBLK, :] = o[0].astype(np.float32)
        attn2[b, 0, r * QBLK : (r + 1) * QBLK, :] = o[1].astype(np.float32)
    return (attn1, attn2)
